# revision 11
# baseline (speedup 1.0000x reference)
"""PixelContrastLoss (supervised contrastive loss) on 8 Trainium2 cores.

Problem: feats [8192, 1, 256] f32, labels [8192] int32 (2 classes).
  f = feats[:,0,:];  logits = f @ f.T / T;  row-stabilized softmax-style loss
  over same-label positives vs different-label negatives; scalar f32 mean.

Strategy (anchor/row sharded):
  * Host sorts rows by label (loss is permutation invariant) and pre-scales f
    by 1/sqrt(T). Columns use the same order, so the label mask becomes two
    contiguous column segments split at n0.
  * The 64 row-blocks (128 rows each) are dealt to (core, slot) so that each
    slot index has a fixed class across all cores: slots [0,s0) are pure
    class-0 blocks, slots (s0,8) pure class-1, and slot s0 is "mixed-capable"
    (handles the boundary block on one core, pure leftovers elsewhere via
    per-partition scale/bias selects). One SPMD program serves all cores.
  * Device, per block: logits [128, 8192] via fp32r matmuls (full-rate fp32);
    exp(l - diag) straight from PSUM with per-partition bias (-diag is the
    row max for this distribution; the shift cancels exactly), accumulated
    per segment by the activation accumulator; ln(E + ns) second ACT pass
    over the positive segment only (both segments with selects on the mixed
    slot); row-sums of raw logits reduced on the vector engine from PSUM.
  * Host combines per-row scalars (O(N)): per-row loss, the reference's NaN
    semantics (neg_sum underflows to 0 with negatives present -> 0*inf ->
    NaN), and the final mean. Only [6, 128, 8] scalars per core leave the
    device; the 8192^2 logits never touch HBM.
"""

import math

import numpy as np

TEMPERATURE = 0.07
BASE_TEMPERATURE = 0.07
N = 8192
D = 256
CORES = 8
P = 128
R = N // CORES          # rows per core
B = R // P              # row-block slots per core
NBLK = N // P           # global row blocks
CH = 1024               # PSUM chunk width (2 banks)
MMN = 512               # matmul moving-dim tile

_compiled = {}


def _plan(n0):
    """Deal the 64 global row-blocks to (core, slot).

    Returns (slot_modes, assign) where slot_modes[s] in 'A','B','M' and
    assign[c][s] = global block index.
    """
    p0 = n0 // P                       # pure class-0 blocks
    mixed = [n0 // P] if n0 % P else []
    p1_start = p0 + len(mixed)
    l0 = list(range(p0))
    l1 = list(range(p1_start, NBLK))
    s0 = len(l0) // CORES
    s1 = len(l1) // CORES
    lm = l0[s0 * CORES:] + mixed + l1[s1 * CORES:]
    slot_modes = ["A"] * s0 + (["M"] if lm else []) + ["B"] * s1
    assert len(slot_modes) == B and len(lm) in (0, CORES)
    assign = []
    for c in range(CORES):
        row = []
        for s in range(B):
            mode = slot_modes[s]
            if mode == "A":
                a_slot = s
                row.append(l0[a_slot * CORES + c])
            elif mode == "B":
                b_slot = s - (s0 + (1 if lm else 0))
                row.append(l1[b_slot * CORES + c])
            else:
                row.append(lm[c])
        assign.append(row)
    return slot_modes, assign


def _pin_act_tables():
    """Make exp/ln resolvable only via natural_log_exp_and_others so the
    table chooser emits a single load instead of thrashing between the
    exp-only and ln-only sets. Set order/indices are preserved."""
    import concourse.bacc as bacc
    import concourse.hw_specs as hw_specs
    from concourse import mybir
    if getattr(bacc, "_act_tables_pinned", False):
        return
    orig = hw_specs.get_activation_tables
    AF = mybir.ActivationFunctionType

    def patched(module_arch):
        tables = orig(module_arch)
        for name, fns in tables.items():
            if name != "natural_log_exp_and_others":
                fns.discard(AF.Exp)
                fns.discard(AF.Ln)
        return tables

    bacc.get_activation_tables = patched
    bacc._act_tables_pinned = True


def _build(n0, skip=(), repeats=1):
    import concourse.bacc as bacc
    import concourse.tile as tile
    from concourse import mybir

    _pin_act_tables()

    F32 = mybir.dt.float32
    F32R = mybir.dt.float32r
    AF = mybir.ActivationFunctionType
    AX = mybir.AxisListType

    W = N
    slot_modes, _ = _plan(n0)

    # column spans: chunks of CH split at the class boundary n0
    spans = []
    for c in range(W // CH):
        lo, hi = c * CH, (c + 1) * CH
        if lo < n0 < hi:
            spans.append((lo, n0, True))
            spans.append((n0, hi, False))
        else:
            spans.append((lo, hi, hi <= n0))
    nA = sum(1 for s in spans if s[2])
    nS = len(spans)

    nc = bacc.Bacc()
    ftl = nc.dram_tensor("ftl", [D, W + R], F32R, kind="ExternalInput")
    fl = nc.dram_tensor("fl", [R, D], F32, kind="ExternalInput")
    clsd = nc.dram_tensor("cls", [R], F32, kind="ExternalInput")
    cls0d = nc.dram_tensor("cls0", [R], F32, kind="ExternalInput")
    dg_o = nc.dram_tensor("dg_o", [P, B], F32, kind="ExternalOutput")
    nsa_o = nc.dram_tensor("nsa_o", [P, B], F32, kind="ExternalOutput")
    nsb_o = nc.dram_tensor("nsb_o", [P, B], F32, kind="ExternalOutput")
    sg_o = nc.dram_tensor("sg_o", [P, B], F32, kind="ExternalOutput")
    xsa_o = nc.dram_tensor("xsa_o", [P, B], F32, kind="ExternalOutput")
    xsb_o = nc.dram_tensor("xsb_o", [P, B], F32, kind="ExternalOutput")

    ftl_r = ftl.rearrange("(c p) n -> p c n", p=P)

    with tile.TileContext(nc) as tc:
        with (
            tc.tile_pool(name="const", bufs=1) as const,
            tc.tile_pool(name="ebuf", bufs=2) as ebuf,
            tc.tile_pool(name="small", bufs=4) as small,
            tc.tile_pool(name="ps", bufs=4, space="PSUM") as ps,
        ):
            ftl_sb = const.tile([P, 2, W + R], F32R)
            # local (lhsT) region first so matmuls can start immediately,
            # then the big column matrix in chunks
            nc.sync.dma_start(out=ftl_sb[:, :, W:W + R], in_=ftl_r[:, :, W:W + R])
            DCH = 1024
            for c in range(W // DCH):
                nc.sync.dma_start(
                    out=ftl_sb[:, :, c * DCH:(c + 1) * DCH],
                    in_=ftl_r[:, :, c * DCH:(c + 1) * DCH])
            fl_sb = const.tile([P, B, D], F32)
            nc.sync.dma_start(out=fl_sb, in_=fl.rearrange("(b p) d -> p b d", p=P))
            clsv = const.tile([P, B], F32)
            nc.sync.dma_start(out=clsv, in_=clsd.rearrange("(b p) -> p b", p=P))
            cls0v = const.tile([P, B], F32)
            nc.sync.dma_start(out=cls0v, in_=cls0d.rearrange("(b p) -> p b", p=P))

            dg = const.tile([P, B], F32)
            for b in range(B):
                sq_junk = small.tile([P, D], F32, tag="sqj")
                nc.scalar.activation(out=sq_junk, in_=fl_sb[:, b, :],
                                     func=AF.Square, accum_out=dg[:, b:b + 1])
            ndg = const.tile([P, B], F32)
            nc.scalar.mul(out=ndg, in_=dg, mul=-1.0)

            rep_ctx = tc.For_i(0, repeats, 1) if repeats > 1 else None
            if rep_ctx is not None:
                rep_ctx.__enter__()

            nsa_all = const.tile([P, B], F32)
            nsb_all = const.tile([P, B], F32)
            sg_all = const.tile([P, B], F32)
            xsa_all = const.tile([P, B], F32)
            xsb_all = const.tile([P, B], F32)

            def make_ln_pieces(b, mode, E, nsA, nsB):
                """Return a list of closures emitting ln(E + ns) pieces for
                block b (deferred so they interleave with the next block's
                chunks). Each piece accumulates into sg_parts; a final
                closure reduces into sg_all."""
                if "ln" in skip or "exp" in skip:
                    return []
                pieces = []
                sg_parts = small.tile([P, 4], F32, tag="sgp")

                def seg_ops(lo_, hi_, bias, scale, j0):
                    mids = []
                    bounds = [lo_] + mids + [hi_]
                    out = []
                    for j in range(len(bounds) - 1):
                        s_, e_ = bounds[j], bounds[j + 1]
                        if s_ == e_:
                            continue
                        def op(s_=s_, e_=e_, j_=j0 + j):
                            nc.scalar.activation(
                                out=E[:, s_:e_], in_=E[:, s_:e_], func=AF.Ln,
                                bias=bias, scale=scale,
                                accum_out=sg_parts[:, j_:j_ + 1])
                        out.append(op)
                    return out

                nseg = 0
                if mode == "A":
                    pieces += seg_ops(0, n0, nsB, 1.0, 0)
                    nseg = len(pieces)
                elif mode == "B":
                    pieces += seg_ops(n0, W, nsA, 1.0, 0)
                    nseg = len(pieces)
                else:
                    if n0 > 0:
                        biasA = small.tile([P, 1], F32, tag="biasA")
                        nc.vector.tensor_mul(out=biasA, in0=nsB,
                                             in1=cls0v[:, b:b + 1])
                        nc.vector.tensor_add(out=biasA, in0=biasA,
                                             in1=clsv[:, b:b + 1])
                        pieces += seg_ops(0, n0, biasA, cls0v[:, b:b + 1], 0)
                    if n0 < W:
                        biasB = small.tile([P, 1], F32, tag="biasB")
                        nc.vector.tensor_mul(out=biasB, in0=nsA,
                                             in1=clsv[:, b:b + 1])
                        nc.vector.tensor_add(out=biasB, in0=biasB,
                                             in1=cls0v[:, b:b + 1])
                        pieces += seg_ops(n0, W, biasB, clsv[:, b:b + 1],
                                          len(pieces))
                    nseg = len(pieces)

                def fin(nseg=nseg):
                    nc.vector.reduce_sum(out=sg_all[:, b:b + 1],
                                         in_=sg_parts[:, 0:nseg], axis=AX.X)
                pieces.append(fin)
                return pieces

            pending = []           # deferred Ln closures from previous block
            for b in range(B):
                mode = slot_modes[b]
                lhs_lo = W + b * P
                E = ebuf.tile([P, W], F32, tag="E")
                ns_parts = small.tile([P, nS], F32, tag="nsp")
                xs_parts = small.tile([P, nS], F32, tag="xsp")

                need_ns = {"A": (False, True), "B": (True, False),
                           "M": (True, True)}[mode]
                need_xs = {"A": (True, False), "B": (False, True),
                           "M": (True, True)}[mode]

                nchunks = W // CH
                for c in range(nchunks):
                    lo = c * CH
                    pt = ps.tile([P, CH], F32, tag="pt")
                    for n in range(CH // MMN):
                        s = lo + n * MMN
                        for k in range(2):
                            nc.tensor.matmul(
                                pt[:, n * MMN:(n + 1) * MMN],
                                ftl_sb[:, k, lhs_lo:lhs_lo + P],
                                ftl_sb[:, k, s:s + MMN],
                                start=(k == 0), stop=(k == 1),
                            )
                    for i, (s, e, isA) in enumerate(spans):
                        if not (lo <= s < lo + CH):
                            continue
                        if "exp" not in skip:
                            acc = (ns_parts[:, i:i + 1]
                                   if need_ns[0 if isA else 1] else None)
                            nc.scalar.activation(
                                out=E[:, s:e], in_=pt[:, s - lo:e - lo],
                                func=AF.Exp, bias=ndg[:, b:b + 1], scale=1.0,
                                accum_out=acc)
                        if "xs" not in skip and need_xs[0 if isA else 1]:
                            nc.vector.reduce_sum(
                                out=xs_parts[:, i:i + 1],
                                in_=pt[:, s - lo:e - lo], axis=AX.X)
                    # interleave a deferred Ln piece of the previous block
                    if pending:
                        pending.pop(0)()

                while pending:
                    pending.pop(0)()

                nsA = small.tile([P, 1], F32, tag="nsA")
                nsB = small.tile([P, 1], F32, tag="nsB")
                if need_ns[0] and "exp" not in skip:
                    if nA > 0:
                        nc.vector.reduce_sum(out=nsA, in_=ns_parts[:, 0:nA],
                                             axis=AX.X)
                    else:
                        nc.vector.memset(nsA, 0.0)
                    nc.vector.tensor_copy(out=nsa_all[:, b:b + 1], in_=nsA)
                if need_ns[1] and "exp" not in skip:
                    if nS > nA:
                        nc.vector.reduce_sum(out=nsB, in_=ns_parts[:, nA:nS],
                                             axis=AX.X)
                    else:
                        nc.vector.memset(nsB, 0.0)
                    nc.vector.tensor_copy(out=nsb_all[:, b:b + 1], in_=nsB)
                if "xs" not in skip:
                    if need_xs[0] and nA > 0:
                        nc.vector.reduce_sum(out=xsa_all[:, b:b + 1],
                                             in_=xs_parts[:, 0:nA], axis=AX.X)
                    elif need_xs[0]:
                        nc.vector.memset(xsa_all[:, b:b + 1], 0.0)
                    if need_xs[1] and nS > nA:
                        nc.vector.reduce_sum(out=xsb_all[:, b:b + 1],
                                             in_=xs_parts[:, nA:nS], axis=AX.X)
                    elif need_xs[1]:
                        nc.vector.memset(xsb_all[:, b:b + 1], 0.0)

                pending = make_ln_pieces(b, mode, E, nsA, nsB)

            while pending:
                pending.pop(0)()

            if rep_ctx is not None:
                rep_ctx.__exit__(None, None, None)

            nc.sync.dma_start(out=dg_o[:, :], in_=dg)
            nc.sync.dma_start(out=nsa_o[:, :], in_=nsa_all)
            nc.sync.dma_start(out=nsb_o[:, :], in_=nsb_all)
            nc.sync.dma_start(out=sg_o[:, :], in_=sg_all)
            nc.sync.dma_start(out=xsa_o[:, :], in_=xsa_all)
            nc.sync.dma_start(out=xsb_o[:, :], in_=xsb_all)

    nc.compile()
    return nc


def _get_program(n0, repeats=1):
    key = (n0, repeats)
    if key not in _compiled:
        _compiled[key] = _build(n0, repeats=repeats)
    return _compiled[key]


def make_in_maps(feats, labels):
    """Shard/stage the full inputs for the 8 cores."""
    f = feats[:, 0, :]
    classes = np.unique(labels)
    assert len(classes) <= 2, "kernel supports 2 label classes"
    if len(classes) == 1:
        lab01 = np.zeros(N, dtype=np.int64)
    else:
        lab01 = (labels == classes[1]).astype(np.int64)
    order = np.argsort(lab01, kind="stable")
    n0 = int((lab01 == 0).sum())

    scale = np.float32(1.0 / math.sqrt(TEMPERATURE))
    fp = (f[order] * scale).astype(np.float32)      # sorted, pre-scaled
    ftp = np.ascontiguousarray(fp.T)                # [D, N]
    labp = lab01[order]

    _, assign = _plan(n0)
    in_maps = []
    rows_by_core = []
    for c in range(CORES):
        rows = np.concatenate(
            [np.arange(g * P, (g + 1) * P) for g in assign[c]])
        rows_by_core.append(rows)
        flc = np.ascontiguousarray(fp[rows])                   # [R, D]
        ftl = np.ascontiguousarray(
            np.concatenate([ftp, flc.T], axis=1))              # [D, N + R]
        clsc = labp[rows].astype(np.float32)
        in_maps.append({
            "ftl": ftl,
            "fl": flc,
            "cls": np.ascontiguousarray(clsc),
            "cls0": np.ascontiguousarray(1.0 - clsc),
        })
    return in_maps, order, n0, lab01, rows_by_core


def assemble(results, order, n0, lab01, rows_by_core):
    """Combine per-row device scalars into the final loss (reference-faithful,
    including its NaN semantics)."""
    n1 = N - n0

    def gather(name):
        vec = np.empty(N, np.float32)
        for c in range(CORES):
            vec[rows_by_core[c]] = results[c][name].T.ravel()
        return vec

    dg = gather("dg_o")
    nsa = gather("nsa_o")
    nsb = gather("nsb_o")
    sg = gather("sg_o")
    xsa = gather("xsa_o")
    xsb = gather("xsb_o")

    cls = lab01[order].astype(bool)
    with np.errstate(divide="ignore", invalid="ignore", over="ignore"):
        ns = np.where(cls, nsa, nsb).astype(np.float32)
        xs = np.where(cls, xsb, xsa).astype(np.float32)
        npos = np.where(cls, np.float32(n1), np.float32(n0))
        nneg = np.where(cls, np.float32(n0), np.float32(n1))
        x_sum = (xs - npos * dg).astype(np.float32)
        g_ii = np.log1p(ns).astype(np.float32)
        lp_sum = (x_sum - (sg - g_ii)).astype(np.float32)
        cnt = (npos - 1.0).astype(np.float32)
        loss_rows = (-(lp_sum / cnt)).astype(np.float32)
        bad = (ns == 0.0) & (nneg > 0)
        loss_rows = np.where(bad, np.float32(np.nan), loss_rows)
        return np.float32(np.mean(loss_rows.astype(np.float32)))


def kernel(feats, labels):
    from concourse.bass_utils import run_bass_kernel_spmd
    feats = np.ascontiguousarray(np.asarray(feats), dtype=np.float32)
    labels = np.asarray(labels)
    assert feats.shape == (N, 1, D), feats.shape
    assert labels.shape == (N,), labels.shape

    in_maps, order, n0, lab01, rows_by_core = make_in_maps(feats, labels)
    nc = _get_program(n0)
    res = run_bass_kernel_spmd(nc, in_maps, core_ids=list(range(CORES)))
    return assemble(res.results, order, n0, lab01, rows_by_core)


# revision 14
# speedup vs baseline: 1.5185x; 1.5185x over previous
"""PixelContrastLoss (supervised contrastive loss) on 8 Trainium2 cores.

Problem: feats [8192, 1, 256] f32, labels [8192] int32 (2 classes).
  f = feats[:,0,:];  logits = f @ f.T / T;  row-stabilized softmax-style loss
  over same-label positives vs different-label negatives; scalar f32 mean.

Strategy (anchor/row sharded):
  * Host sorts rows by label (loss is permutation invariant) and pre-scales f
    by 1/sqrt(T). Columns use the same order, so the label mask becomes two
    contiguous column segments split at n0.
  * The 64 row-blocks (128 rows each) are dealt to (core, slot) so that each
    slot index has a fixed class across all cores: slots [0,s0) are pure
    class-0 blocks, slots (s0,8) pure class-1, and slot s0 is "mixed-capable"
    (handles the boundary block on one core, pure leftovers elsewhere via
    per-partition scale/bias selects). One SPMD program serves all cores.
  * Device, per block: logits [128, 8192] via fp32r matmuls (full-rate fp32);
    exp(l - diag) straight from PSUM with per-partition bias (-diag is the
    row max for this distribution; the shift cancels exactly), accumulated
    per segment by the activation accumulator; ln(E + ns) second ACT pass
    over the positive segment only (both segments with selects on the mixed
    slot); row-sums of raw logits reduced on the vector engine from PSUM.
  * Host combines per-row scalars (O(N)): per-row loss, the reference's NaN
    semantics (neg_sum underflows to 0 with negatives present -> 0*inf ->
    NaN), and the final mean. Only [6, 128, 8] scalars per core leave the
    device; the 8192^2 logits never touch HBM.
"""

import math

import numpy as np

TEMPERATURE = 0.07
BASE_TEMPERATURE = 0.07
N = 8192
D = 256
CORES = 8
P = 128
R = N // CORES          # rows per core
B = R // P              # row-block slots per core
NBLK = N // P           # global row blocks
CH = 1024               # PSUM chunk width (2 banks)
MMN = 512               # matmul moving-dim tile

_compiled = {}


def _plan(n0):
    """Deal the 64 global row-blocks to (core, slot).

    Returns (slot_modes, assign) where slot_modes[s] in 'A','B','M' and
    assign[c][s] = global block index.
    """
    p0 = n0 // P                       # pure class-0 blocks
    mixed = [n0 // P] if n0 % P else []
    p1_start = p0 + len(mixed)
    l0 = list(range(p0))
    l1 = list(range(p1_start, NBLK))
    s0 = len(l0) // CORES
    s1 = len(l1) // CORES
    lm = l0[s0 * CORES:] + mixed + l1[s1 * CORES:]
    slot_modes = ["A"] * s0 + (["M"] if lm else []) + ["B"] * s1
    assert len(slot_modes) == B and len(lm) in (0, CORES)
    assign = []
    for c in range(CORES):
        row = []
        for s in range(B):
            mode = slot_modes[s]
            if mode == "A":
                a_slot = s
                row.append(l0[a_slot * CORES + c])
            elif mode == "B":
                b_slot = s - (s0 + (1 if lm else 0))
                row.append(l1[b_slot * CORES + c])
            else:
                row.append(lm[c])
        assign.append(row)
    return slot_modes, assign


def _pin_act_tables():
    """Make exp/ln resolvable only via natural_log_exp_and_others so the
    table chooser emits a single load instead of thrashing between the
    exp-only and ln-only sets. Set order/indices are preserved."""
    import concourse.bacc as bacc
    import concourse.hw_specs as hw_specs
    from concourse import mybir
    if getattr(bacc, "_act_tables_pinned", False):
        return
    orig = hw_specs.get_activation_tables
    AF = mybir.ActivationFunctionType

    def patched(module_arch):
        tables = orig(module_arch)
        for name, fns in tables.items():
            if name != "natural_log_exp_and_others":
                fns.discard(AF.Exp)
                fns.discard(AF.Ln)
        return tables

    bacc.get_activation_tables = patched
    bacc._act_tables_pinned = True


def _build(n0, skip=(), repeats=1):
    import concourse.bacc as bacc
    import concourse.tile as tile
    from concourse import mybir

    _pin_act_tables()

    F32 = mybir.dt.float32
    F32R = mybir.dt.float32r
    AF = mybir.ActivationFunctionType
    AX = mybir.AxisListType

    W = N
    slot_modes, _ = _plan(n0)

    # column spans: chunks of CH split at the class boundary n0
    spans = []
    for c in range(W // CH):
        lo, hi = c * CH, (c + 1) * CH
        if lo < n0 < hi:
            spans.append((lo, n0, True))
            spans.append((n0, hi, False))
        else:
            spans.append((lo, hi, hi <= n0))
    nA = sum(1 for s in spans if s[2])
    nS = len(spans)

    nc = bacc.Bacc()
    ftl = nc.dram_tensor("ftl", [D, W + R + 2], F32R, kind="ExternalInput")
    ndgd = nc.dram_tensor("ndg", [R], F32, kind="ExternalInput")
    clsd = nc.dram_tensor("cls", [R], F32, kind="ExternalInput")
    cls0d = nc.dram_tensor("cls0", [R], F32, kind="ExternalInput")
    dg_o = nc.dram_tensor("dg_o", [P, B], F32, kind="ExternalOutput")
    nsa_o = nc.dram_tensor("nsa_o", [P, B], F32, kind="ExternalOutput")
    nsb_o = nc.dram_tensor("nsb_o", [P, B], F32, kind="ExternalOutput")
    sg_o = nc.dram_tensor("sg_o", [P, B], F32, kind="ExternalOutput")
    xsa_o = nc.dram_tensor("xsa_o", [P, B], F32, kind="ExternalOutput")
    xsb_o = nc.dram_tensor("xsb_o", [P, B], F32, kind="ExternalOutput")

    ftl_r = ftl.rearrange("(c p) n -> p c n", p=P)

    with tile.TileContext(nc) as tc:
        with (
            tc.tile_pool(name="const", bufs=1) as const,
            tc.tile_pool(name="ebuf", bufs=3) as ebuf,
            tc.tile_pool(name="small", bufs=4) as small,
            tc.tile_pool(name="ps", bufs=3, space="PSUM") as ps,
            tc.tile_pool(name="psg", bufs=2, space="PSUM") as psg,
        ):
            ftl_sb = const.tile([P, 2, W + R + 2], F32R)
            # local (lhsT) region first so matmuls can start immediately,
            # then the big column matrix in chunks
            nc.sync.dma_start(out=ftl_sb[:, :, W:W + R + 2],
                              in_=ftl_r[:, :, W:W + R + 2])
            DCH = 1024
            for c in range(W // DCH):
                eng = nc.sync if c % 2 == 0 else nc.gpsimd
                eng.dma_start(
                    out=ftl_sb[:, :, c * DCH:(c + 1) * DCH],
                    in_=ftl_r[:, :, c * DCH:(c + 1) * DCH])
            ndg = const.tile([P, B], F32)
            nc.sync.dma_start(out=ndg, in_=ndgd.rearrange("(b p) -> p b", p=P))
            clsv = const.tile([P, B], F32)
            nc.sync.dma_start(out=clsv, in_=clsd.rearrange("(b p) -> p b", p=P))
            cls0v = const.tile([P, B], F32)
            nc.sync.dma_start(out=cls0v, in_=cls0d.rearrange("(b p) -> p b", p=P))


            rep_ctx = tc.For_i(0, repeats, 1) if repeats > 1 else None
            if rep_ctx is not None:
                rep_ctx.__enter__()

            nsa_all = const.tile([P, B], F32)
            nsb_all = const.tile([P, B], F32)
            sg_all = const.tile([P, B], F32)
            xsa_all = const.tile([P, B], F32)
            xsb_all = const.tile([P, B], F32)

            def make_ln_pieces(b, mode, E, nsA, nsB):
                """Return a list of closures emitting ln(E + ns) pieces for
                block b (deferred so they interleave with the next block's
                chunks). Each piece accumulates into sg_parts; a final
                closure reduces into sg_all."""
                if "ln" in skip or "exp" in skip:
                    return []
                pieces = []
                sg_parts = small.tile([P, 4], F32, tag="sgp")

                def seg_ops(lo_, hi_, bias, scale, j0):
                    mids = []
                    bounds = [lo_] + mids + [hi_]
                    out = []
                    for j in range(len(bounds) - 1):
                        s_, e_ = bounds[j], bounds[j + 1]
                        if s_ == e_:
                            continue
                        def op(s_=s_, e_=e_, j_=j0 + j):
                            nc.scalar.activation(
                                out=E[:, s_:e_], in_=E[:, s_:e_], func=AF.Ln,
                                bias=bias, scale=scale,
                                accum_out=sg_parts[:, j_:j_ + 1])
                        out.append(op)
                    return out

                nseg = 0
                if mode == "A":
                    pieces += seg_ops(0, n0, nsB, 1.0, 0)
                    nseg = len(pieces)
                elif mode == "B":
                    pieces += seg_ops(n0, W, nsA, 1.0, 0)
                    nseg = len(pieces)
                else:
                    if n0 > 0:
                        biasA = small.tile([P, 1], F32, tag="biasA")
                        nc.vector.tensor_mul(out=biasA, in0=nsB,
                                             in1=cls0v[:, b:b + 1])
                        nc.vector.tensor_add(out=biasA, in0=biasA,
                                             in1=clsv[:, b:b + 1])
                        pieces += seg_ops(0, n0, biasA, cls0v[:, b:b + 1], 0)
                    if n0 < W:
                        biasB = small.tile([P, 1], F32, tag="biasB")
                        nc.vector.tensor_mul(out=biasB, in0=nsA,
                                             in1=clsv[:, b:b + 1])
                        nc.vector.tensor_add(out=biasB, in0=biasB,
                                             in1=cls0v[:, b:b + 1])
                        pieces += seg_ops(n0, W, biasB, clsv[:, b:b + 1],
                                          len(pieces))
                    nseg = len(pieces)

                def fin(nseg=nseg):
                    nc.vector.reduce_sum(out=sg_all[:, b:b + 1],
                                         in_=sg_parts[:, 0:nseg], axis=AX.X)
                pieces.append(fin)
                return pieces

            pending = []           # deferred Ln closures from previous block
            for b in range(B):
                mode = slot_modes[b]
                lhs_lo = W + b * P
                E = ebuf.tile([P, W], F32, tag="E")

                need_ns = {"A": (False, True), "B": (True, False),
                           "M": (True, True)}[mode]

                # row-sums of raw logits over each class segment via a tiny
                # matmul against the class-sum vectors g (cols W+R..W+R+2)
                gp = psg.tile([P, 2], F32, tag="gp")
                for k in range(2):
                    nc.tensor.matmul(
                        gp, ftl_sb[:, k, lhs_lo:lhs_lo + P],
                        ftl_sb[:, k, W + R:W + R + 2],
                        start=(k == 0), stop=(k == 1))
                nc.vector.tensor_copy(out=xsa_all[:, b:b + 1], in_=gp[:, 0:1])
                nc.vector.tensor_copy(out=xsb_all[:, b:b + 1], in_=gp[:, 1:2])

                nchunks = W // CH
                for c in range(nchunks):
                    lo = c * CH
                    pt = ps.tile([P, CH], F32, tag="pt")
                    for n in range(CH // MMN):
                        s = lo + n * MMN
                        for k in range(2):
                            nc.tensor.matmul(
                                pt[:, n * MMN:(n + 1) * MMN],
                                ftl_sb[:, k, lhs_lo:lhs_lo + P],
                                ftl_sb[:, k, s:s + MMN],
                                start=(k == 0), stop=(k == 1),
                            )
                    # vector engine drains PSUM into the SBUF row buffer
                    nc.vector.tensor_copy(out=E[:, lo:lo + CH], in_=pt)
                    # interleave a deferred Ln piece of the previous block
                    if pending:
                        pending.pop(0)()

                while pending:
                    pending.pop(0)()

                # exp(l - diag) in place, one op per class segment, with the
                # negative-side sum accumulated directly
                nsA = small.tile([P, 1], F32, tag="nsA")
                nsB = small.tile([P, 1], F32, tag="nsB")
                if "exp" not in skip:
                    if n0 > 0:
                        nc.scalar.activation(
                            out=E[:, 0:n0], in_=E[:, 0:n0], func=AF.Exp,
                            bias=ndg[:, b:b + 1], scale=1.0,
                            accum_out=nsA if need_ns[0] else None)
                    elif need_ns[0]:
                        nc.vector.memset(nsA, 0.0)
                    if n0 < W:
                        nc.scalar.activation(
                            out=E[:, n0:W], in_=E[:, n0:W], func=AF.Exp,
                            bias=ndg[:, b:b + 1], scale=1.0,
                            accum_out=nsB if need_ns[1] else None)
                    elif need_ns[1]:
                        nc.vector.memset(nsB, 0.0)
                    if need_ns[0]:
                        nc.vector.tensor_copy(out=nsa_all[:, b:b + 1], in_=nsA)
                    if need_ns[1]:
                        nc.vector.tensor_copy(out=nsb_all[:, b:b + 1], in_=nsB)

                pending = make_ln_pieces(b, mode, E, nsA, nsB)

            while pending:
                pending.pop(0)()

            if rep_ctx is not None:
                rep_ctx.__exit__(None, None, None)

            dgout = small.tile([P, B], F32, tag="dgo")
            nc.vector.tensor_scalar_mul(dgout, ndg, -1.0)
            nc.sync.dma_start(out=dg_o[:, :], in_=dgout)
            nc.sync.dma_start(out=nsa_o[:, :], in_=nsa_all)
            nc.sync.dma_start(out=nsb_o[:, :], in_=nsb_all)
            nc.sync.dma_start(out=sg_o[:, :], in_=sg_all)
            nc.sync.dma_start(out=xsa_o[:, :], in_=xsa_all)
            nc.sync.dma_start(out=xsb_o[:, :], in_=xsb_all)

    nc.compile()
    return nc


def _get_program(n0, repeats=1):
    key = (n0, repeats)
    if key not in _compiled:
        _compiled[key] = _build(n0, repeats=repeats)
    return _compiled[key]


def make_in_maps(feats, labels):
    """Shard/stage the full inputs for the 8 cores."""
    f = feats[:, 0, :]
    classes = np.unique(labels)
    assert len(classes) <= 2, "kernel supports 2 label classes"
    if len(classes) == 1:
        lab01 = np.zeros(N, dtype=np.int64)
    else:
        lab01 = (labels == classes[1]).astype(np.int64)
    order = np.argsort(lab01, kind="stable")
    n0 = int((lab01 == 0).sum())

    scale = np.float32(1.0 / math.sqrt(TEMPERATURE))
    fp = (f[order] * scale).astype(np.float32)      # sorted, pre-scaled
    ftp = np.ascontiguousarray(fp.T)                # [D, N]
    labp = lab01[order]

    _, assign = _plan(n0)
    in_maps = []
    rows_by_core = []
    for c in range(CORES):
        rows = np.concatenate(
            [np.arange(g * P, (g + 1) * P) for g in assign[c]])
        rows_by_core.append(rows)
        flc = np.ascontiguousarray(fp[rows])                   # [R, D]
        gA = fp[:n0].sum(0, dtype=np.float32).reshape(D, 1)
        gB = fp[n0:].sum(0, dtype=np.float32).reshape(D, 1)
        ftl = np.ascontiguousarray(
            np.concatenate([ftp, flc.T, gA, gB], axis=1))      # [D, N + R + 2]
        clsc = labp[rows].astype(np.float32)
        ndgc = -(flc.astype(np.float32) ** 2).sum(1, dtype=np.float32)
        in_maps.append({
            "ftl": ftl,
            "ndg": np.ascontiguousarray(ndgc.astype(np.float32)),
            "cls": np.ascontiguousarray(clsc),
            "cls0": np.ascontiguousarray(1.0 - clsc),
        })
    return in_maps, order, n0, lab01, rows_by_core


def assemble(results, order, n0, lab01, rows_by_core):
    """Combine per-row device scalars into the final loss (reference-faithful,
    including its NaN semantics)."""
    n1 = N - n0

    def gather(name):
        vec = np.empty(N, np.float32)
        for c in range(CORES):
            vec[rows_by_core[c]] = results[c][name].T.ravel()
        return vec

    dg = gather("dg_o")
    nsa = gather("nsa_o")
    nsb = gather("nsb_o")
    sg = gather("sg_o")
    xsa = gather("xsa_o")
    xsb = gather("xsb_o")

    cls = lab01[order].astype(bool)
    with np.errstate(divide="ignore", invalid="ignore", over="ignore"):
        ns = np.where(cls, nsa, nsb).astype(np.float32)
        xs = np.where(cls, xsb, xsa).astype(np.float32)
        npos = np.where(cls, np.float32(n1), np.float32(n0))
        nneg = np.where(cls, np.float32(n0), np.float32(n1))
        x_sum = (xs - npos * dg).astype(np.float32)
        g_ii = np.log1p(ns).astype(np.float32)
        lp_sum = (x_sum - (sg - g_ii)).astype(np.float32)
        cnt = (npos - 1.0).astype(np.float32)
        loss_rows = (-(lp_sum / cnt)).astype(np.float32)
        bad = (ns == 0.0) & (nneg > 0)
        loss_rows = np.where(bad, np.float32(np.nan), loss_rows)
        return np.float32(np.mean(loss_rows.astype(np.float32)))


def kernel(feats, labels):
    from concourse.bass_utils import run_bass_kernel_spmd
    feats = np.ascontiguousarray(np.asarray(feats), dtype=np.float32)
    labels = np.asarray(labels)
    assert feats.shape == (N, 1, D), feats.shape
    assert labels.shape == (N,), labels.shape

    in_maps, order, n0, lab01, rows_by_core = make_in_maps(feats, labels)
    nc = _get_program(n0)
    res = run_bass_kernel_spmd(nc, in_maps, core_ids=list(range(CORES)))
    return assemble(res.results, order, n0, lab01, rows_by_core)


# revision 17
# speedup vs baseline: 2.0143x; 1.3265x over previous
"""PixelContrastLoss (supervised contrastive loss) on 8 Trainium2 cores.

Problem: feats [8192, 1, 256] f32, labels [8192] int32 (2 classes).
  f = feats[:,0,:];  logits = f @ f.T / T;  row-stabilized softmax-style loss
  over same-label positives vs different-label negatives; scalar f32 mean.

Strategy (anchor/row sharded):
  * Host sorts rows by label (loss is permutation invariant) and pre-scales f
    by 1/sqrt(T). Columns use the same order, so the label mask becomes two
    contiguous column segments split at n0.
  * The 64 row-blocks (128 rows each) are dealt to (core, slot) so that each
    slot index has a fixed class across all cores: slots [0,s0) are pure
    class-0 blocks, slots (s0,8) pure class-1, and slot s0 is "mixed-capable"
    (handles the boundary block on one core, pure leftovers elsewhere via
    per-partition scale/bias selects). One SPMD program serves all cores.
  * Device, per block: logits [128, 8192] via fp32r matmuls (full-rate fp32);
    exp(l - diag) straight from PSUM with per-partition bias (-diag is the
    row max for this distribution; the shift cancels exactly), accumulated
    per segment by the activation accumulator; ln(E + ns) second ACT pass
    over the positive segment only (both segments with selects on the mixed
    slot); row-sums of raw logits reduced on the vector engine from PSUM.
  * Host combines per-row scalars (O(N)): per-row loss, the reference's NaN
    semantics (neg_sum underflows to 0 with negatives present -> 0*inf ->
    NaN), and the final mean. Only [6, 128, 8] scalars per core leave the
    device; the 8192^2 logits never touch HBM.
"""

import math

import numpy as np

TEMPERATURE = 0.07
BASE_TEMPERATURE = 0.07
N = 8192
D = 256
CORES = 8
P = 128
R = N // CORES          # rows per core
B = R // P              # row-block slots per core
NBLK = N // P           # global row blocks
CH = 1024               # PSUM chunk width (2 banks)
MMN = 512               # matmul moving-dim tile

_compiled = {}


def _plan(n0):
    """Deal the 64 global row-blocks to (core, slot).

    Returns (slot_modes, assign) where slot_modes[s] in 'A','B','M' and
    assign[c][s] = global block index.
    """
    p0 = n0 // P                       # pure class-0 blocks
    mixed = [n0 // P] if n0 % P else []
    p1_start = p0 + len(mixed)
    l0 = list(range(p0))
    l1 = list(range(p1_start, NBLK))
    s0 = len(l0) // CORES
    s1 = len(l1) // CORES
    lm = l0[s0 * CORES:] + mixed + l1[s1 * CORES:]
    slot_modes = ["A"] * s0 + (["M"] if lm else []) + ["B"] * s1
    assert len(slot_modes) == B and len(lm) in (0, CORES)
    assign = []
    for c in range(CORES):
        row = []
        for s in range(B):
            mode = slot_modes[s]
            if mode == "A":
                a_slot = s
                row.append(l0[a_slot * CORES + c])
            elif mode == "B":
                b_slot = s - (s0 + (1 if lm else 0))
                row.append(l1[b_slot * CORES + c])
            else:
                row.append(lm[c])
        assign.append(row)
    return slot_modes, assign


def _pin_act_tables():
    """Make exp/ln resolvable only via natural_log_exp_and_others so the
    table chooser emits a single load instead of thrashing between the
    exp-only and ln-only sets. Set order/indices are preserved."""
    import concourse.bacc as bacc
    import concourse.hw_specs as hw_specs
    from concourse import mybir
    if getattr(bacc, "_act_tables_pinned", False):
        return
    orig = hw_specs.get_activation_tables
    AF = mybir.ActivationFunctionType

    def patched(module_arch):
        tables = orig(module_arch)
        for name, fns in tables.items():
            if name != "natural_log_exp_and_others":
                fns.discard(AF.Exp)
                fns.discard(AF.Ln)
        return tables

    bacc.get_activation_tables = patched
    bacc._act_tables_pinned = True


def _build(n0, skip=(), repeats=1):
    import concourse.bacc as bacc
    import concourse.tile as tile
    from concourse import mybir

    _pin_act_tables()

    F32 = mybir.dt.float32
    F32R = mybir.dt.float32r
    AF = mybir.ActivationFunctionType
    AX = mybir.AxisListType

    W = N
    slot_modes, _ = _plan(n0)

    # column spans: chunks of CH split at the class boundary n0
    spans = []
    for c in range(W // CH):
        lo, hi = c * CH, (c + 1) * CH
        if lo < n0 < hi:
            spans.append((lo, n0, True))
            spans.append((n0, hi, False))
        else:
            spans.append((lo, hi, hi <= n0))
    nA = sum(1 for s in spans if s[2])
    nS = len(spans)

    nc = bacc.Bacc()
    ftl = nc.dram_tensor("ftl", [D, W + R + 2], F32R, kind="ExternalInput")
    ndgd = nc.dram_tensor("ndg", [R], F32, kind="ExternalInput")
    clsd = nc.dram_tensor("cls", [R], F32, kind="ExternalInput")
    cls0d = nc.dram_tensor("cls0", [R], F32, kind="ExternalInput")
    dg_o = nc.dram_tensor("dg_o", [P, B], F32, kind="ExternalOutput")
    nsa_o = nc.dram_tensor("nsa_o", [P, B], F32, kind="ExternalOutput")
    nsb_o = nc.dram_tensor("nsb_o", [P, B], F32, kind="ExternalOutput")
    sg_o = nc.dram_tensor("sg_o", [P, B], F32, kind="ExternalOutput")
    xsa_o = nc.dram_tensor("xsa_o", [P, B], F32, kind="ExternalOutput")
    xsb_o = nc.dram_tensor("xsb_o", [P, B], F32, kind="ExternalOutput")

    ftl_r = ftl.rearrange("(c p) n -> p c n", p=P)

    with tile.TileContext(nc) as tc:
        with (
            tc.tile_pool(name="const", bufs=1) as const,
            tc.tile_pool(name="ebuf", bufs=3) as ebuf,
            tc.tile_pool(name="small", bufs=6) as small,
            tc.tile_pool(name="ps", bufs=3, space="PSUM") as ps,
            tc.tile_pool(name="psg", bufs=2, space="PSUM") as psg,
        ):
            ftl_sb = const.tile([P, 2, W + R + 2], F32R)
            # local (lhsT) region first so matmuls can start immediately,
            # then the big column matrix in chunks
            nc.sync.dma_start(out=ftl_sb[:, :, W:W + R + 2],
                              in_=ftl_r[:, :, W:W + R + 2])
            DCH = 1024
            for c in range(W // DCH):
                eng = nc.sync if c % 3 == 0 else nc.gpsimd
                eng.dma_start(
                    out=ftl_sb[:, :, c * DCH:(c + 1) * DCH],
                    in_=ftl_r[:, :, c * DCH:(c + 1) * DCH])
            ndg = const.tile([P, B], F32)
            nc.sync.dma_start(out=ndg, in_=ndgd.rearrange("(b p) -> p b", p=P))
            clsv = const.tile([P, B], F32)
            nc.sync.dma_start(out=clsv, in_=clsd.rearrange("(b p) -> p b", p=P))
            cls0v = const.tile([P, B], F32)
            nc.sync.dma_start(out=cls0v, in_=cls0d.rearrange("(b p) -> p b", p=P))


            rep_ctx = tc.For_i(0, repeats, 1) if repeats > 1 else None
            if rep_ctx is not None:
                rep_ctx.__enter__()

            nsa_all = const.tile([P, B], F32)
            nsb_all = const.tile([P, B], F32)
            sg_all = const.tile([P, B], F32)
            xsa_all = const.tile([P, B], F32)
            xsb_all = const.tile([P, B], F32)

            def make_ln_pieces(b, mode, E, nsA, nsB):
                """Return a list of closures emitting ln(E + ns) pieces for
                block b (deferred so they interleave with the next block's
                chunks). Each piece accumulates into sg_parts; a final
                closure reduces into sg_all."""
                if "ln" in skip or "exp" in skip:
                    return []
                pieces = []
                sg_parts = small.tile([P, 4], F32, tag="sgp")

                def seg_ops(lo_, hi_, bias, scale, j0):
                    mids = []
                    bounds = [lo_] + mids + [hi_]
                    out = []
                    for j in range(len(bounds) - 1):
                        s_, e_ = bounds[j], bounds[j + 1]
                        if s_ == e_:
                            continue
                        def op(s_=s_, e_=e_, j_=j0 + j):
                            nc.scalar.activation(
                                out=E[:, s_:e_], in_=E[:, s_:e_], func=AF.Ln,
                                bias=bias, scale=scale,
                                accum_out=sg_parts[:, j_:j_ + 1])
                        out.append(op)
                    return out

                nseg = 0
                if mode == "A":
                    pieces += seg_ops(0, n0, nsB, 1.0, 0)
                    nseg = len(pieces)
                elif mode == "B":
                    pieces += seg_ops(n0, W, nsA, 1.0, 0)
                    nseg = len(pieces)
                else:
                    if n0 > 0:
                        biasA = small.tile([P, 1], F32, tag="biasA")
                        nc.vector.tensor_mul(out=biasA, in0=nsB,
                                             in1=cls0v[:, b:b + 1])
                        nc.vector.tensor_add(out=biasA, in0=biasA,
                                             in1=clsv[:, b:b + 1])
                        pieces += seg_ops(0, n0, biasA, cls0v[:, b:b + 1], 0)
                    if n0 < W:
                        biasB = small.tile([P, 1], F32, tag="biasB")
                        nc.vector.tensor_mul(out=biasB, in0=nsA,
                                             in1=clsv[:, b:b + 1])
                        nc.vector.tensor_add(out=biasB, in0=biasB,
                                             in1=cls0v[:, b:b + 1])
                        pieces += seg_ops(n0, W, biasB, clsv[:, b:b + 1],
                                          len(pieces))
                    nseg = len(pieces)

                def fin(nseg=nseg):
                    nc.vector.reduce_sum(out=sg_all[:, b:b + 1],
                                         in_=sg_parts[:, 0:nseg], axis=AX.X)
                pieces.append(fin)
                return pieces

            pending = []           # deferred Ln closures from previous block
            for b in range(B):
                mode = slot_modes[b]
                lhs_lo = W + b * P
                E = ebuf.tile([P, W], F32, tag="E")

                need_ns = {"A": (False, True), "B": (True, False),
                           "M": (True, True)}[mode]

                # row-sums of raw logits over each class segment via a tiny
                # matmul against the class-sum vectors g (cols W+R..W+R+2)
                gp = psg.tile([P, 2], F32, tag="gp")
                for k in range(2):
                    nc.tensor.matmul(
                        gp, ftl_sb[:, k, lhs_lo:lhs_lo + P],
                        ftl_sb[:, k, W + R:W + R + 2],
                        start=(k == 0), stop=(k == 1))
                nc.vector.tensor_copy(out=xsa_all[:, b:b + 1], in_=gp[:, 0:1])
                nc.vector.tensor_copy(out=xsb_all[:, b:b + 1], in_=gp[:, 1:2])

                nchunks = W // CH
                for c in range(nchunks):
                    lo = c * CH
                    pt = ps.tile([P, CH], F32, tag="pt")
                    for n in range(CH // MMN):
                        s = lo + n * MMN
                        for k in range(2):
                            nc.tensor.matmul(
                                pt[:, n * MMN:(n + 1) * MMN],
                                ftl_sb[:, k, lhs_lo:lhs_lo + P],
                                ftl_sb[:, k, s:s + MMN],
                                start=(k == 0), stop=(k == 1),
                            )
                    # vector engine drains PSUM into the SBUF row buffer
                    nc.vector.tensor_copy(out=E[:, lo:lo + CH], in_=pt)
                    # interleave a deferred Ln piece of the previous block
                    if pending:
                        pending.pop(0)()

                while pending:
                    pending.pop(0)()

                # exp(l - diag) in place, one op per class segment, with the
                # negative-side sum accumulated directly
                nsA = small.tile([P, 1], F32, tag="nsA")
                nsB = small.tile([P, 1], F32, tag="nsB")
                if "exp" not in skip:
                    if n0 > 0:
                        nc.scalar.activation(
                            out=E[:, 0:n0], in_=E[:, 0:n0], func=AF.Exp,
                            bias=ndg[:, b:b + 1], scale=1.0,
                            accum_out=nsA if need_ns[0] else None)
                    elif need_ns[0]:
                        nc.vector.memset(nsA, 0.0)
                    if n0 < W:
                        nc.scalar.activation(
                            out=E[:, n0:W], in_=E[:, n0:W], func=AF.Exp,
                            bias=ndg[:, b:b + 1], scale=1.0,
                            accum_out=nsB if need_ns[1] else None)
                    elif need_ns[1]:
                        nc.vector.memset(nsB, 0.0)
                    if need_ns[0]:
                        nc.vector.tensor_copy(out=nsa_all[:, b:b + 1], in_=nsA)
                    if need_ns[1]:
                        nc.vector.tensor_copy(out=nsb_all[:, b:b + 1], in_=nsB)

                pending = make_ln_pieces(b, mode, E, nsA, nsB)

            while pending:
                pending.pop(0)()

            if rep_ctx is not None:
                rep_ctx.__exit__(None, None, None)

            dgout = small.tile([P, B], F32, tag="dgo")
            nc.vector.tensor_scalar_mul(dgout, ndg, -1.0)
            nc.sync.dma_start(out=dg_o[:, :], in_=dgout)
            nc.sync.dma_start(out=nsa_o[:, :], in_=nsa_all)
            nc.sync.dma_start(out=nsb_o[:, :], in_=nsb_all)
            nc.sync.dma_start(out=sg_o[:, :], in_=sg_all)
            nc.sync.dma_start(out=xsa_o[:, :], in_=xsa_all)
            nc.sync.dma_start(out=xsb_o[:, :], in_=xsb_all)

    nc.compile()
    return nc


def _get_program(n0, repeats=1):
    key = (n0, repeats)
    if key not in _compiled:
        _compiled[key] = _build(n0, repeats=repeats)
    return _compiled[key]


def make_in_maps(feats, labels):
    """Shard/stage the full inputs for the 8 cores."""
    f = feats[:, 0, :]
    classes = np.unique(labels)
    assert len(classes) <= 2, "kernel supports 2 label classes"
    if len(classes) == 1:
        lab01 = np.zeros(N, dtype=np.int64)
    else:
        lab01 = (labels == classes[1]).astype(np.int64)
    order = np.argsort(lab01, kind="stable")
    n0 = int((lab01 == 0).sum())

    scale = np.float32(1.0 / math.sqrt(TEMPERATURE))
    fp = (f[order] * scale).astype(np.float32)      # sorted, pre-scaled
    ftp = np.ascontiguousarray(fp.T)                # [D, N]
    labp = lab01[order]

    _, assign = _plan(n0)
    in_maps = []
    rows_by_core = []
    for c in range(CORES):
        rows = np.concatenate(
            [np.arange(g * P, (g + 1) * P) for g in assign[c]])
        rows_by_core.append(rows)
        flc = np.ascontiguousarray(fp[rows])                   # [R, D]
        gA = fp[:n0].sum(0, dtype=np.float32).reshape(D, 1)
        gB = fp[n0:].sum(0, dtype=np.float32).reshape(D, 1)
        ftl = np.ascontiguousarray(
            np.concatenate([ftp, flc.T, gA, gB], axis=1))      # [D, N + R + 2]
        clsc = labp[rows].astype(np.float32)
        ndgc = -(flc.astype(np.float32) ** 2).sum(1, dtype=np.float32)
        in_maps.append({
            "ftl": ftl,
            "ndg": np.ascontiguousarray(ndgc.astype(np.float32)),
            "cls": np.ascontiguousarray(clsc),
            "cls0": np.ascontiguousarray(1.0 - clsc),
        })
    return in_maps, order, n0, lab01, rows_by_core


def assemble(results, order, n0, lab01, rows_by_core):
    """Combine per-row device scalars into the final loss (reference-faithful,
    including its NaN semantics)."""
    n1 = N - n0

    def gather(name):
        vec = np.empty(N, np.float32)
        for c in range(CORES):
            vec[rows_by_core[c]] = results[c][name].T.ravel()
        return vec

    dg = gather("dg_o")
    nsa = gather("nsa_o")
    nsb = gather("nsb_o")
    sg = gather("sg_o")
    xsa = gather("xsa_o")
    xsb = gather("xsb_o")

    cls = lab01[order].astype(bool)
    with np.errstate(divide="ignore", invalid="ignore", over="ignore"):
        ns = np.where(cls, nsa, nsb).astype(np.float32)
        xs = np.where(cls, xsb, xsa).astype(np.float32)
        npos = np.where(cls, np.float32(n1), np.float32(n0))
        nneg = np.where(cls, np.float32(n0), np.float32(n1))
        x_sum = (xs - npos * dg).astype(np.float32)
        g_ii = np.log1p(ns).astype(np.float32)
        lp_sum = (x_sum - (sg - g_ii)).astype(np.float32)
        cnt = (npos - 1.0).astype(np.float32)
        loss_rows = (-(lp_sum / cnt)).astype(np.float32)
        bad = (ns == 0.0) & (nneg > 0)
        loss_rows = np.where(bad, np.float32(np.nan), loss_rows)
        return np.float32(np.mean(loss_rows.astype(np.float32)))


def kernel(feats, labels):
    from concourse.bass_utils import run_bass_kernel_spmd
    feats = np.ascontiguousarray(np.asarray(feats), dtype=np.float32)
    labels = np.asarray(labels)
    assert feats.shape == (N, 1, D), feats.shape
    assert labels.shape == (N,), labels.shape

    in_maps, order, n0, lab01, rows_by_core = make_in_maps(feats, labels)
    nc = _get_program(n0)
    res = run_bass_kernel_spmd(nc, in_maps, core_ids=list(range(CORES)))
    return assemble(res.results, order, n0, lab01, rows_by_core)
